# revision 22
# baseline (speedup 1.0000x reference)
"""Trainium2 Bass kernel for nn_ExpandedResolventFMNet.

Mathematical reformulation (validated in fp64 against the jax reference):

The reference builds kron(A.T, My) [8192x4096], its Gram [4096^2], resolvent
kron masks, and solves a dense 4096x4096 system.  All of that collapses:

  first        = kron(A A^T, G),              G = My^T My
  second       = kron-sum of 64x64 factors; with X = Mx W the full system is
  M(W)         = S~ W G + LMBDA * sum_d Dd*( (Dd*W) G ) = R~    (* = Hadamard)
  S~           = Mx^T (A A^T) Mx
  R~           = Mx^T A Bc^T My
  Dd           = resolvent-mask difference matrices (64x64)
  output C     = (Mx W)^T

The 4096x4096 operator kron(S~,G)+LMBDA*blockdiag is SPD with cond ~3e2; PCG
with the exact-kron preconditioner P^-1 = kron(S~^-1, G^-1) (applied as two
64x64 matmuls) converges to the fp32 floor in <=14 iterations.  The device
runs the transposed system in Y = W^T:

  M'(Y) = G Y S~ + sum_d DdT * (G (DdT * Y)),   C = Y Mx^T

and exploits symmetry so that every matmul is transpose-free:
  (G p)^T   = mm(lhsT=p,  rhs=G)     [G symmetric]
  (G p) S~  = mm(lhsT=(G p)^T, rhs=S~)
  (Gi r)^T  = mm(lhsT=r,  rhs=Gi)
  (Gi r) Si = mm(lhsT=(Gi r)^T, rhs=Si)

S~^-1 and G^-1 are produced on-device by Newton-Schulz iteration.
sqrt(LMBDA) is folded into DdT.  Work is sharded over 8 cores for the
V=5000 projections (AllReduce of the 64KB partials); the small solve runs
redundantly on every core.
"""

import numpy as np

import concourse.bacc as bacc
import concourse.mybir as mybir
from concourse.bass_isa import ReduceOp
from concourse.bass_utils import run_bass_kernel_spmd
from concourse.masks import make_identity
from concourse.tile import TileContext

F32 = mybir.dt.float32
K = 64          # spectral basis size
C = 128         # feature channels
V = 5000        # vertices
CHUNK = 125     # v-contraction tile (partition dim)
N_CORES = 8
N_ITERS = 12
NEWTON_STEPS_S = 8
NEWTON_STEPS_G = 4
SQRT_LMBDA = 10.0

SHARD = True    # shard projections over cores + AllReduce partials

_PROGRAM_CACHE = {}


def build_program(shard: bool):
    nc = bacc.Bacc("TRN2", num_devices=N_CORES)
    v_local = V // N_CORES if shard else V          # 625 or 5000
    n_chunks = v_local // CHUNK                     # 5 or 40

    fx_d = nc.dram_tensor("fx", [v_local, C], F32, kind="ExternalInput")
    fy_d = nc.dram_tensor("fy", [v_local, C], F32, kind="ExternalInput")
    pxT_d = nc.dram_tensor("pxT", [v_local, K], F32, kind="ExternalInput")
    pyT_d = nc.dram_tensor("pyT", [v_local, K], F32, kind="ExternalInput")
    mx_d = nc.dram_tensor("mx", [K, K], F32, kind="ExternalInput")
    my_d = nc.dram_tensor("my", [K, K], F32, kind="ExternalInput")
    mxT_d = nc.dram_tensor("mxT", [K, K], F32, kind="ExternalInput")
    myT_d = nc.dram_tensor("myT", [K, K], F32, kind="ExternalInput")
    ev_d = nc.dram_tensor("ev", [1, 2 * K], F32, kind="ExternalInput")
    out_d = nc.dram_tensor("out", [K, K], F32, kind="ExternalOutput")

    if shard:
        ccx_in = nc.dram_tensor("ccx_in", [C, K], F32)
        ccx_out = nc.dram_tensor("ccx_out", [C, K], F32, addr_space="Shared")
        ccy_in = nc.dram_tensor("ccy_in", [C, K], F32)
        ccy_out = nc.dram_tensor("ccy_out", [C, K], F32, addr_space="Shared")

    with TileContext(nc) as tc:
        with (
            tc.tile_pool(name="big", bufs=1) as bp,
            tc.tile_pool(name="persist", bufs=1) as sp,
            tc.tile_pool(name="work", bufs=2) as wp,
            tc.tile_pool(name="psum", bufs=2, space="PSUM") as pp,
        ):

            # rotating psum tags: 3 tags x bufs=2 -> 6 banks (+proj acc 2 = 8)
            _ps_state = {"i": 0}

            def ps_tile(shape):
                i = _ps_state["i"]
                _ps_state["i"] += 1
                return pp.tile(shape, F32, tag=f"ps{i % 3}", name=f"pst{i}")

            def sb_copy(src_psum, shape, pool, tag, engine="vector"):
                t = pool.tile(shape, F32, tag=tag, name=tag)
                if engine == "vector":
                    nc.vector.tensor_copy(t, src_psum)
                else:
                    nc.scalar.copy(t, src_psum)
                return t

            # ---------------- input DMA (one DMA per big tensor) ------------
            fx_t = bp.tile([CHUNK, n_chunks, C], F32)
            fy_t = bp.tile([CHUNK, n_chunks, C], F32)
            pxT_t = bp.tile([CHUNK, n_chunks, K], F32)
            pyT_t = bp.tile([CHUNK, n_chunks, K], F32)
            nc.sync.dma_start(
                fx_t, fx_d.rearrange("(n p) c -> p n c", p=CHUNK))
            nc.sync.dma_start(
                fy_t, fy_d.rearrange("(n p) c -> p n c", p=CHUNK))
            nc.sync.dma_start(
                pxT_t, pxT_d.rearrange("(n p) c -> p n c", p=CHUNK))
            nc.sync.dma_start(
                pyT_t, pyT_d.rearrange("(n p) c -> p n c", p=CHUNK))
            mx_s = sp.tile([K, K], F32)
            my_s = sp.tile([K, K], F32)
            mxT_s = sp.tile([K, K], F32)
            myT_s = sp.tile([K, K], F32)
            ev_t = sp.tile([1, 2 * K], F32)
            nc.sync.dma_start(mx_s, mx_d[:, :])
            nc.sync.dma_start(my_s, my_d[:, :])
            nc.sync.dma_start(mxT_s, mxT_d[:, :])
            nc.sync.dma_start(myT_s, myT_d[:, :])
            nc.sync.dma_start(ev_t, ev_d[:, :])

            ident = sp.tile([C, C], F32)
            make_identity(nc, ident)
            id64 = ident[0:K, 0:K]
            ones_row = sp.tile([1, K], F32)
            nc.vector.memset(ones_row, 1.0)
            ones_col = sp.tile([K, 1], F32)
            nc.vector.memset(ones_col, 1.0)


            # ---------------- projections: AT = fx^T pxT, ByT = fy^T pyT ----
            with tc.tile_pool(name="pacc", bufs=1, space="PSUM") as pacc:
                at_p = pacc.tile([C, K], F32)    # A^T partial  [C,K]
                byt_p = pacc.tile([C, K], F32)   # By^T partial [C,K]
                for n in range(n_chunks):
                    nc.tensor.matmul(at_p, fx_t[:, n, :], pxT_t[:, n, :],
                                     start=(n == 0), stop=(n == n_chunks - 1))
                if shard:
                    # x-side collective issues while the y-side projections run
                    partx_s = sp.tile([C, K], F32)
                    nc.vector.tensor_copy(partx_s, at_p)
                    nc.sync.dma_start(ccx_in[:, :], partx_s)
                    nc.gpsimd.collective_compute(
                        "AllReduce", mybir.AluOpType.add,
                        replica_groups=[list(range(N_CORES))],
                        ins=[ccx_in[:, :]], outs=[ccx_out[:, :]])
                for n in range(n_chunks):
                    nc.tensor.matmul(byt_p, fy_t[:, n, :], pyT_t[:, n, :],
                                     start=(n == 0), stop=(n == n_chunks - 1))
                if shard:
                    party_s = sp.tile([C, K], F32)
                    nc.vector.tensor_copy(party_s, byt_p)
                    nc.sync.dma_start(ccy_in[:, :], party_s)
                    nc.gpsimd.collective_compute(
                        "AllReduce", mybir.AluOpType.add,
                        replica_groups=[list(range(N_CORES))],
                        ins=[ccy_in[:, :]], outs=[ccy_out[:, :]])
                else:
                    at_s = sb_copy(at_p, [C, K], sp, "at_s")
                    byt_s = sb_copy(byt_p, [C, K], sp, "byt_s")

            # ------- collective-independent work first (hides CC latency) ---
            # G = My^T My
            g_p = ps_tile([K, K])
            nc.tensor.matmul(g_p, my_s, my_s)
            g_s = sb_copy(g_p, [K, K], sp, "g_s")

            # resolvent masks: ev = [ex | ey]; t = ev/max(ev); im = 1/(1+t);
            # re = sqrt(t)*im; both scaled by sqrt(LMBDA)
            evmax = sp.tile([1, 1], F32)
            nc.vector.tensor_reduce(evmax, ev_t, mybir.AxisListType.X,
                                    mybir.AluOpType.max)
            evrec = sp.tile([1, 1], F32)
            nc.vector.reciprocal(evrec, evmax)
            t_t = sp.tile([1, 2 * K], F32)
            nc.vector.tensor_scalar_mul(t_t, ev_t, evrec)
            tp1 = sp.tile([1, 2 * K], F32)
            nc.vector.tensor_scalar_add(tp1, t_t, 1.0)
            im_t = sp.tile([1, 2 * K], F32)
            nc.vector.reciprocal(im_t, tp1)
            sq_t = sp.tile([1, 2 * K], F32)
            nc.scalar.sqrt(sq_t, t_t)
            re_t = sp.tile([1, 2 * K], F32)
            nc.vector.tensor_mul(re_t, sq_t, im_t)
            nc.vector.tensor_scalar_mul(re_t, re_t, SQRT_LMBDA)
            nc.vector.tensor_scalar_mul(im_t, im_t, SQRT_LMBDA)

            # D1T[a,i] = re2[a] - re1[i]; D2T likewise from im
            d_s = []
            for idx, src in enumerate((re_t, im_t)):
                pa = ps_tile([K, K])
                nc.tensor.matmul(pa, src[0:1, K:2 * K], ones_row)  # v2[p]
                pb = ps_tile([K, K])
                nc.tensor.matmul(pb, ones_row, src[0:1, 0:K])      # v1[f]
                ta = sb_copy(pa, [K, K], sp, f"dta{idx}")
                dt = sp.tile([K, K], F32, tag=f"d{idx}t_s", name=f"d{idx}t_s")
                nc.vector.tensor_sub(dt, ta, pb)
                d_s.append(dt)
            d1t_s, d2t_s = d_s
            d12t_s = sp.tile([K, 2 * K], F32)
            nc.vector.tensor_copy(d12t_s[:, 0:K], d1t_s)
            nc.vector.tensor_copy(d12t_s[:, K:2 * K], d2t_s)

            # Newton-Schulz inverse (S symmetric PD): X' = 2X - X S X
            def newton_inverse(mat_s, tag, steps):
                rs = sp.tile([K, 1], F32, tag=f"{tag}_rs", name=f"{tag}_rs")
                nc.vector.tensor_reduce(rs, mat_s, mybir.AxisListType.X,
                                        mybir.AluOpType.add,
                                        apply_absolute_value=True)
                nc.gpsimd.partition_all_reduce(rs, rs, K, ReduceOp.max)
                al = sp.tile([K, 1], F32, tag=f"{tag}_al", name=f"{tag}_al")
                nc.vector.reciprocal(al, rs)
                x_s = sp.tile([K, K], F32, tag=f"{tag}_x0", name=f"{tag}_x0")
                nc.vector.tensor_scalar_mul(x_s, id64, al)
                for it in range(steps):
                    t1 = ps_tile([K, K])
                    nc.tensor.matmul(t1, mat_s, x_s)          # S X (S sym)
                    t1s = wp.tile([K, K], F32, tag=f"{tag}_t1s",
                                  name=f"{tag}_t1s")
                    nc.vector.tensor_copy(t1s, t1)
                    t2 = ps_tile([K, K])
                    nc.tensor.matmul(t2, x_s, t1s)            # X (S X) (X sym)
                    xn = sp.tile([K, K], F32, tag=f"{tag}_x{it + 1}",
                                 name=f"{tag}_x{it + 1}")
                    nc.vector.scalar_tensor_tensor(
                        xn, x_s, 2.0, t2,
                        op0=mybir.AluOpType.mult,
                        op1=mybir.AluOpType.subtract)
                    x_s = xn
                return x_s

            gi_s = newton_inverse(g_s, "gi", NEWTON_STEPS_G)

            # ------- collective-dependent chain -----------------------------
            if shard:
                at_s = sp.tile([C, K], F32, tag="at_s", name="at_s")
                nc.sync.dma_start(at_s, ccx_out[:, :])

            # S~ = Mx^T (A A^T) Mx    [S_A symmetric -> no transpose]
            sa_p = ps_tile([K, K])
            nc.tensor.matmul(sa_p, at_s, at_s)          # A A^T
            sa_s = sb_copy(sa_p, [K, K], sp, "sa_s")
            h1t_p = ps_tile([K, K])
            nc.tensor.matmul(h1t_p, sa_s, mx_s)         # S_A Mx (sym trick)
            h1t_s = sb_copy(h1t_p, [K, K], sp, "h1t_s")
            st_p = ps_tile([K, K])
            nc.tensor.matmul(st_p, mx_s, h1t_s)         # Mx^T S_A Mx
            st_s = sb_copy(st_p, [K, K], sp, "st_s")

            si_s = newton_inverse(st_s, "si", NEWTON_STEPS_S)

            if shard:
                byt_s = sp.tile([C, K], F32, tag="byt_s", name="byt_s")
                nc.sync.dma_start(byt_s, ccy_out[:, :])

            # RHS' = My^T Bc A^T Mx = My^T (My (By A^T)) Mx
            byat_p = ps_tile([K, K])
            nc.tensor.matmul(byat_p, byt_s, at_s)       # By A^T
            byat_s = sb_copy(byat_p, [K, K], sp, "byat_s")
            bca_p = ps_tile([K, K])
            nc.tensor.matmul(bca_p, myT_s, byat_s)      # My (By A^T) = Bc A^T
            bca_s = sb_copy(bca_p, [K, K], sp, "bca_s")
            w_p = ps_tile([K, K])
            nc.tensor.matmul(w_p, my_s, bca_s)          # My^T Bc A^T
            w_s = sb_copy(w_p, [K, K], sp, "w_s")
            wt_p = ps_tile([K, K])
            nc.tensor.transpose(wt_p, w_s, id64)
            wt_s = sb_copy(wt_p, [K, K], sp, "wt_s")
            rp_p = ps_tile([K, K])
            nc.tensor.matmul(rp_p, wt_s, mx_s)          # (My^T Bc A^T) Mx
            r_s = sp.tile([K, K], F32)                  # CG residual
            nc.vector.tensor_copy(r_s, rp_p)

            # ------- PCG: pipelined (vector recurrences, exact dots) --------
            # state: y, r, z=P^-1 r, p, q=Mp, s=P^-1 q; per iteration the
            # matvec w=Mz and precond v=P^-1 w run concurrently with the
            # dot/axpy chain; p,q,s advance by the beta-recurrence.
            y_s = sp.tile([K, K], F32)
            nc.vector.memset(y_s, 0.0)
            p_s = sp.tile([K, K], F32)
            q_s = sp.tile([K, K], F32)
            s_s = sp.tile([K, K], F32)
            z_s = sp.tile([K, K], F32)
            u_s = sp.tile([K, 2 * K], F32)   # stacked [D1T*z | D2T*z]

            def precond_psum(x_tile, tag):
                """P^-1 x in PSUM via (Gi x)^T = mm(lhsT=x, rhs=Gi)."""
                ut_p = ps_tile([K, K])
                nc.tensor.matmul(ut_p, x_tile, gi_s)
                ut_s = wp.tile([K, K], F32, tag=f"{tag}_uts", name=f"{tag}_uts")
                nc.scalar.copy(ut_s, ut_p)
                v_p = ps_tile([K, K])
                nc.tensor.matmul(v_p, ut_s, si_s)
                return v_p

            def matvec_z(tag):
                """w = M z into SBUF (reads z_s)."""
                nc.vector.tensor_mul(u_s[:, 0:K], d1t_s, z_s)
                nc.vector.tensor_mul(u_s[:, K:2 * K], d2t_s, z_s)
                gzt_p = ps_tile([K, K])
                nc.tensor.matmul(gzt_p, z_s, g_s)         # (G z)^T
                gzt_s = wp.tile([K, K], F32, tag="mv_gzts", name="mv_gzts")
                nc.scalar.copy(gzt_s, gzt_p)
                t2_p = ps_tile([K, K])
                nc.tensor.matmul(t2_p, gzt_s, st_s)       # (G z) S~
                gu_p = ps_tile([K, 2 * K])
                nc.tensor.matmul(gu_p[:, 0:K], g_s, u_s[:, 0:K])   # G u1
                nc.tensor.matmul(gu_p[:, K:2 * K], g_s, u_s[:, K:2 * K])
                mm_s = wp.tile([K, 2 * K], F32, tag="mv_mm", name="mv_mm")
                nc.vector.tensor_mul(mm_s, d12t_s, gu_p)  # masked, both halves
                a1_s = wp.tile([K, K], F32, tag="mv_a1", name="mv_a1")
                nc.vector.tensor_add(a1_s, mm_s[:, 0:K], t2_p)
                w_s = wp.tile([K, K], F32, tag="mv_w", name="mv_w")
                nc.vector.tensor_add(w_s, a1_s, mm_s[:, K:2 * K])
                return w_s

            def dot_b(a_ap, b_ap, tag):
                """<a,b> broadcast to all partitions as [K,1] SBUF."""
                prod = wp.tile([K, K], F32, tag="dot_dm", name="dot_dm")
                acc = wp.tile([K, 1], F32, tag=f"{tag}_acc", name=f"{tag}_acc")
                nc.vector.scalar_tensor_tensor(
                    prod, a_ap, 1.0, b_ap,
                    op0=mybir.AluOpType.bypass, op1=mybir.AluOpType.mult,
                    accum_out=acc)
                nc.gpsimd.partition_all_reduce(acc, acc, K, ReduceOp.add)
                return acc

            # init: z = P^-1 r; w = Mz; v = P^-1 w; p=z, q=w, s=v
            z0_p = precond_psum(r_s, "pcz")
            nc.vector.tensor_copy(z_s, z0_p)
            nc.vector.tensor_copy(p_s, z0_p)
            rz0 = dot_b(r_s, z_s, "rz")
            rzrec = wp.tile([K, 1], F32, tag="rzrec", name="rzrec")
            nc.vector.reciprocal(rzrec, rz0)
            rzneg = wp.tile([K, 1], F32, tag="rzneg", name="rzneg")
            nc.vector.tensor_scalar_mul(rzneg, rz0, -1.0)
            w_s = matvec_z("init")
            nc.vector.tensor_copy(q_s, w_s)
            v_p = precond_psum(w_s, "pcv")
            nc.vector.tensor_copy(s_s, v_p)

            for it in range(N_ITERS):
                # ---- alpha = rz/<p,q>; r,z,y updates ----
                pq = dot_b(p_s, q_s, "pq")
                pqr = wp.tile([K, 1], F32, tag="pqr", name="pqr")
                nc.vector.reciprocal(pqr, pq)
                if it < N_ITERS - 1:
                    an = wp.tile([K, 1], F32, tag="an", name="an")
                    nc.vector.tensor_mul(an, rzneg, pqr)
                    nc.vector.scalar_tensor_tensor(
                        r_s, q_s, an, r_s,
                        op0=mybir.AluOpType.mult, op1=mybir.AluOpType.add)
                    nc.vector.scalar_tensor_tensor(
                        z_s, s_s, an, z_s,
                        op0=mybir.AluOpType.mult, op1=mybir.AluOpType.add)
                al = wp.tile([K, 1], F32, tag="al", name="al")
                nc.vector.tensor_mul(al, rz0, pqr)
                nc.vector.scalar_tensor_tensor(
                    y_s, p_s, al, y_s,
                    op0=mybir.AluOpType.mult, op1=mybir.AluOpType.add)

                if it == N_ITERS - 1:
                    break

                # ---- rz_new, beta; w/v for the NEXT q,s updates ----
                rz_new = dot_b(r_s, z_s, "rz")
                w_s = matvec_z(f"i{it}")
                if it < N_ITERS - 2:
                    v_p = precond_psum(w_s, f"pcv")
                bt = wp.tile([K, 1], F32, tag="bt", name="bt")
                nc.vector.tensor_mul(bt, rz_new, rzrec)
                nc.vector.scalar_tensor_tensor(
                    p_s, p_s, bt, z_s,
                    op0=mybir.AluOpType.mult, op1=mybir.AluOpType.add)
                nc.vector.scalar_tensor_tensor(
                    q_s, q_s, bt, w_s,
                    op0=mybir.AluOpType.mult, op1=mybir.AluOpType.add)
                if it < N_ITERS - 2:
                    nc.vector.scalar_tensor_tensor(
                        s_s, s_s, bt, v_p,
                        op0=mybir.AluOpType.mult, op1=mybir.AluOpType.add)
                rz0 = rz_new
                rzrec = wp.tile([K, 1], F32, tag="rzrec", name="rzrec")
                nc.vector.reciprocal(rzrec, rz0)
                rzneg = wp.tile([K, 1], F32, tag="rzneg", name="rzneg")
                nc.vector.tensor_scalar_mul(rzneg, rz0, -1.0)

            # ---------------- output: C = Y Mx^T ----------------
            yt_p = ps_tile([K, K])
            nc.tensor.transpose(yt_p, y_s, id64)
            yt_s = wp.tile([K, K], F32, tag="yt_s", name="yt_s")
            nc.vector.tensor_copy(yt_s, yt_p)
            c_p = ps_tile([K, K])
            nc.tensor.matmul(c_p, yt_s, mxT_s)      # Y Mx^T
            c_s = wp.tile([K, K], F32, tag="c_s", name="c_s")
            nc.vector.tensor_copy(c_s, c_p)
            nc.sync.dma_start(out_d[:, :], c_s)

    nc.finalize()
    return nc


def get_program(shard: bool):
    if shard not in _PROGRAM_CACHE:
        _PROGRAM_CACHE[shard] = build_program(shard)
    return _PROGRAM_CACHE[shard]


def make_in_maps(inputs, shard: bool):
    fx = np.ascontiguousarray(np.asarray(inputs["feat_x"], np.float32)[0])
    fy = np.ascontiguousarray(np.asarray(inputs["feat_y"], np.float32)[0])
    pxT = np.ascontiguousarray(np.asarray(inputs["evecs_trans_x"], np.float32)[0].T)
    pyT = np.ascontiguousarray(np.asarray(inputs["evecs_trans_y"], np.float32)[0].T)
    mx = np.ascontiguousarray(np.asarray(inputs["sqrtMk_x"], np.float32)[0])
    my = np.ascontiguousarray(np.asarray(inputs["sqrtMk_y"], np.float32)[0])
    ev = np.ascontiguousarray(np.concatenate([
        np.asarray(inputs["evals_x"], np.float32)[0],
        np.asarray(inputs["evals_y"], np.float32)[0],
    ])[None, :])
    small = {
        "mx": mx, "my": my,
        "mxT": np.ascontiguousarray(mx.T),
        "myT": np.ascontiguousarray(my.T),
        "ev": ev,
    }
    in_maps = []
    for c in range(N_CORES):
        if shard:
            lo, hi = c * (V // N_CORES), (c + 1) * (V // N_CORES)
            m = {"fx": fx[lo:hi], "fy": fy[lo:hi],
                 "pxT": pxT[lo:hi], "pyT": pyT[lo:hi]}
        else:
            m = {"fx": fx, "fy": fy, "pxT": pxT, "pyT": pyT}
        m.update(small)
        in_maps.append(m)
    return in_maps


def kernel(**inputs) -> np.ndarray:
    nc = get_program(SHARD)
    in_maps = make_in_maps(inputs, SHARD)
    res = run_bass_kernel_spmd(nc, in_maps, core_ids=list(range(N_CORES)))
    out = np.asarray(res.results[0]["out"], dtype=np.float32)
    return out[None]
